# revision 15
# baseline (speedup 1.0000x reference)
"""v10: v2 structure (2 gathers/chunk, DVE assembly, sync writes) with a
graded chunk schedule [256, 512x7, 256]: small head chunk starts the
SDMA engines ~2us earlier after the library reload, small tail chunk makes
the final write 1MB instead of 2MB. Chain cost ~= v2 (16 calls)."""

import numpy as np

import concourse.bacc as bacc
import concourse.mybir as mybir
from concourse.tile import TileContext
from concourse import library_config
from concourse.bass_utils import run_bass_kernel_spmd

B, T, H = 8, 4096, 512
TROWS = T + 2
ZROW = T + 1

CHUNKS = []
_sizes = [256, 512, 512, 512, 512, 512, 512, 512, 256]
assert sum(_sizes) == T
_s = 0
for _n in _sizes:
    CHUNKS.append((_s, _n))
    _s += _n
MAXM = max(n for _, n in CHUNKS) // 128
IDX_COLS = 2 * T // 16

_NC = None


def _build():
    nc = bacc.Bacc("TRN2", target_bir_lowering=False, debug=False)
    f16 = mybir.dt.float16
    x = nc.dram_tensor("x", [TROWS, 2 * H], f16, kind="ExternalInput")
    idx = nc.dram_tensor("idx", [128, IDX_COLS], mybir.dt.int16,
                         kind="ExternalInput")
    out = nc.dram_tensor("out", [T, 4 * H], f16, kind="ExternalOutput")
    nc.gpsimd.load_library(library_config.mlp)
    with TileContext(nc) as tc:
        with (
            tc.tile_pool(name="idxp", bufs=1) as idxp,
            tc.tile_pool(name="gp", bufs=6) as gp,
            tc.tile_pool(name="ap", bufs=6) as ap,
        ):
            idx_t = idxp.tile([128, IDX_COLS], mybir.dt.int16)
            nc.sync.dma_start(idx_t[:], idx[:])
            nregs = {}
            for n in sorted({n for _, n in CHUNKS}):
                nregs[n] = nc.gpsimd.to_reg(n)
            col = 0
            for c, (start, n) in enumerate(CHUNKS):
                m = n // 128
                ncols = n // 16
                g1 = gp.tile([128, MAXM, 2 * H], f16, tag="g1")
                g2 = gp.tile([128, MAXM, 2 * H], f16, tag="g2")
                for tl in (g1, g2):
                    nc.gpsimd.dma_gather(
                        tl[:, 0:m, :], x[:, :], idx_t[:, col:col + ncols],
                        n, nregs[n], 2 * H,
                    )
                    col += ncols
                a = ap.tile([128, MAXM, 4 * H], f16, tag="a")
                nc.vector.tensor_sub(a[:, 0:m, 0:H],
                                     g1[:, 0:m, 0:H], g2[:, 0:m, 0:H])
                nc.vector.tensor_sub(a[:, 0:m, H:2 * H],
                                     g2[:, 0:m, H:2 * H], g1[:, 0:m, H:2 * H])
                nc.vector.tensor_copy(a[:, 0:m, 2 * H:3 * H], g2[:, 0:m, 0:H])
                nc.vector.tensor_copy(a[:, 0:m, 3 * H:4 * H],
                                      g1[:, 0:m, H:2 * H])
                ov = out[start:start + n, :].rearrange("(p m) e -> p m e",
                                                       p=128)
                nc.sync.dma_start(ov, a[:, 0:m, :])
    nc.compile()
    return nc


def _get_nc():
    global _NC
    if _NC is None:
        _NC = _build()
    return _NC


def _make_inputs(input, span_idxs):
    x = np.asarray(input, dtype=np.float32)
    si = np.asarray(span_idxs).astype(np.int64)
    in_maps = []
    for b in range(B):
        xt = np.zeros((TROWS, 2 * H), np.float16)
        xt[1:T + 1, 0:H] = x[b, :, 0:H]
        xt[0:T, H:2 * H] = x[b, :, H:2 * H]
        i = si[b, :, 0]
        j = si[b, :, 1]
        valid = ~((i == 0) & (j == 0))
        k1 = np.where(valid, j + 1, ZROW).astype(np.int16)
        k2 = np.where(valid, i, ZROW).astype(np.int16)
        cols = []
        for start, n in CHUNKS:
            m = n // 128
            spans = start + (np.arange(128)[:, None] * m
                             + np.arange(m)[None, :])     # [128, m]
            for arr in (k1, k2):
                V = arr[spans]                            # [128, m]
                slot = V.T.reshape(-1)                    # slot r*128+p
                w = slot.reshape(n // 16, 16).T           # 16-wrapped
                cols.append(np.tile(w, (8, 1)))
        idxbuf = np.concatenate(cols, axis=1)
        assert idxbuf.shape == (128, IDX_COLS)
        in_maps.append({"x": xt, "idx": idxbuf.astype(np.int16)})
    return in_maps


def kernel(input, span_idxs):
    nc = _get_nc()
    in_maps = _make_inputs(input, span_idxs)
    res = run_bass_kernel_spmd(nc, in_maps, core_ids=list(range(B)))
    return np.stack(
        [res.results[b]["out"].astype(np.float32) for b in range(B)], axis=0
    )


# revision 16
# speedup vs baseline: 1.0192x; 1.0192x over previous
"""MinusSpan Trainium2 kernel (8-core data parallel, fp16 on-device IO).

Reference op (per batch b, span s):
    i, j = span_idxs[b, s]
    f_pre   = fwd[i-1]  (0 if i == 0)         fwd = input[b, :, :512]
    b_post  = bwd[j+1]  (0 if j+1 >= T)       bwd = input[b, :, 512:]
    f_end   = fwd[j];  b_start = bwd[i]
    out[b, s] = concat(f_end - f_pre, b_start - b_post, f_pre, b_post)
    rows with (i, j) == (0, 0) are zero.

Strategy: pure data parallel over batch (8 cores, 1 sequence each).
The host builds a shifted pair table in fp16
    XT[k] = [fwd[k-1] | bwd[k]]   (k = 0..T, fwd[-1] = bwd[T] = 0)
    XT[T+1] = 0                   (zero row for invalid spans)
so each span needs just TWO 2KB-row gathers:
    G1 = XT[j+1] -> [f_end | b_post]      (j+1 >= T edge baked into row T)
    G2 = XT[i]   -> [f_pre | b_start]     (i == 0 edge baked into row 0)
    out = [G1.lo - G2.lo, G2.hi - G1.hi, G2.lo, G1.hi]
Invalid spans index the zero row.

fp16 halves HBM traffic vs the f32 roofline baseline (16MB gathered
reads + 16MB writes per core); the f32 output is reconstructed on the
host (tolerance is rel_err < 2e-2; fp16 keeps it ~6e-4).  Per chunk of
N spans a SINGLE dma_gather fetches G1 rows into free-rows [0, m) and
G2 rows into [m, 2m) of each partition (m = N/128): one gpsimd
descriptor-generation call per chunk amortizes the ~1us fixed Q7 cost
(desc-gen runs at ~8-9ns/row on one Q7 cpu pair and is the second-
longest chain after the DMA itself).  DVE computes the two
subtractions and the Activation engine (otherwise idle) copies the
f_pre / b_post pass-through halves into the same output tile — DVE
tensor_copy measured 5-7us per chunk under SBUF port contention and
made Vector the pipeline pacer.  One 8KB-row write per chunk on the
sync HWDGE ring keeps every DMA descriptor large and contiguous.
Chunk sizes are graded: a small first chunk starts the SDMA engines
earlier (desc-gen can only begin after the ~10us gpsimd ucode library
reload, preloaded right after the entry barrier to overlap the idx
load), and a small last chunk shortens the drain tail.
"""

import numpy as np

import concourse.bacc as bacc
import concourse.mybir as mybir
from concourse.tile import TileContext
from concourse import library_config
from concourse.bass_utils import run_bass_kernel_spmd

B, T, H = 8, 4096, 512
TROWS = T + 2        # shifted pair table rows (zero row at index T+1)
ZROW = T + 1

# (start, size) chunk schedule: small head for early DMA start, small tail
# for a short drain. sum == T, every size a multiple of 128.
CHUNKS = []
_sizes = [128, 512, 512, 512, 512, 512, 512, 512, 256, 128]
assert sum(_sizes) == T
_s = 0
for _n in _sizes:
    CHUNKS.append((_s, _n))
    _s += _n
MAXM = max(n for _, n in CHUNKS) // 128
IDX_COLS = 2 * T // 16   # total idx columns (2 gathers per span, 16-wrapped)

_NC = None


def _build():
    nc = bacc.Bacc("TRN2", target_bir_lowering=False, debug=False)
    f16 = mybir.dt.float16
    x = nc.dram_tensor("x", [TROWS, 2 * H], f16, kind="ExternalInput")
    idx = nc.dram_tensor("idx", [128, IDX_COLS], mybir.dt.int16,
                         kind="ExternalInput")
    out = nc.dram_tensor("out", [T, 4 * H], f16, kind="ExternalOutput")

    # preload the gpsimd ucode library that dma_gather needs right after the
    # entry barrier, so the ~10us Q7 overlay reload overlaps the idx load
    # instead of stalling the first gather (it cannot move before the entry
    # barrier: the preamble's engine-queue DRAIN would fence on the reload
    # and delay every engine)
    nc.gpsimd.load_library(library_config.mlp)

    with TileContext(nc) as tc:
        with (
            tc.tile_pool(name="idxp", bufs=1) as idxp,
            tc.tile_pool(name="gp", bufs=6) as gp,
            tc.tile_pool(name="apool", bufs=6) as apool,
        ):
            idx_t = idxp.tile([128, IDX_COLS], mybir.dt.int16)
            nc.sync.dma_start(idx_t[:], idx[:])
            nregs = {}
            for n in sorted({n for _, n in CHUNKS}):
                nregs[n] = nc.gpsimd.to_reg(2 * n)
            col = 0
            for c, (start, n) in enumerate(CHUNKS):
                m = n // 128
                ncols = 2 * n // 16
                g = gp.tile([128, 2 * MAXM, 2 * H], f16, tag="g")
                nc.gpsimd.dma_gather(
                    g[:, 0:2 * m, :], x[:, :], idx_t[:, col:col + ncols],
                    2 * n, nregs[n], 2 * H,
                )
                col += ncols
                a = apool.tile([128, MAXM, 4 * H], f16, tag="a")
                nc.vector.tensor_sub(a[:, 0:m, 0:H],
                                     g[:, 0:m, 0:H], g[:, m:2 * m, 0:H])
                nc.vector.tensor_sub(a[:, 0:m, H:2 * H],
                                     g[:, m:2 * m, H:2 * H], g[:, 0:m, H:2 * H])
                nc.scalar.copy(a[:, 0:m, 2 * H:3 * H], g[:, m:2 * m, 0:H])
                nc.scalar.copy(a[:, 0:m, 3 * H:4 * H], g[:, 0:m, H:2 * H])
                ov = out[start:start + n, :].rearrange("(p m) e -> p m e",
                                                       p=128)
                nc.sync.dma_start(ov, a[:, 0:m, :])
    nc.compile()
    return nc


def _get_nc():
    global _NC
    if _NC is None:
        _NC = _build()
    return _NC


def _make_inputs(input, span_idxs):
    x = np.asarray(input, dtype=np.float32)
    si = np.asarray(span_idxs).astype(np.int64)
    in_maps = []
    for b in range(B):
        xt = np.zeros((TROWS, 2 * H), np.float16)
        xt[1:T + 1, 0:H] = x[b, :, 0:H]        # fwd[k-1] at row k
        xt[0:T, H:2 * H] = x[b, :, H:2 * H]    # bwd[k] at row k
        i = si[b, :, 0]
        j = si[b, :, 1]
        valid = ~((i == 0) & (j == 0))
        k1 = np.where(valid, j + 1, ZROW).astype(np.int16)
        k2 = np.where(valid, i, ZROW).astype(np.int16)
        cols = []
        for start, n in CHUNKS:
            m = n // 128
            # span of (partition p, local row r) = start + p*m + r;
            # partition p's free-rows [0,m) hold G1, [m,2m) hold G2
            spans = start + (np.arange(128)[:, None] * m
                             + np.arange(m)[None, :])     # [128, m]
            V = np.concatenate([k1[spans], k2[spans]], axis=1)  # [128, 2m]
            # gather slot s = r*128 + p  ->  V[p, r]
            slot = V.T.reshape(-1)                        # [2n]
            w = slot.reshape(2 * n // 16, 16).T           # 16-wrapped
            cols.append(np.tile(w, (8, 1)))               # [128, 2n/16]
        idxbuf = np.concatenate(cols, axis=1)
        assert idxbuf.shape == (128, IDX_COLS)
        in_maps.append({"x": xt, "idx": idxbuf.astype(np.int16)})
    return in_maps


def kernel(input, span_idxs):
    nc = _get_nc()
    in_maps = _make_inputs(input, span_idxs)
    res = run_bass_kernel_spmd(nc, in_maps, core_ids=list(range(B)))
    return np.stack(
        [res.results[b]["out"].astype(np.float32) for b in range(B)], axis=0
    )
